# revision 1
# baseline (speedup 1.0000x reference)
"""Cross-attention kernel for Trainium2 (8 NeuronCores, Bass/Tile).

Problem (hardcoded):
    B=4, S=2048, D=768 fp32.
    img_n/ref_n/pose_n = LayerNorm(x) (shared gamma/beta)
    Q = ref_n @ Wq.T + bq ; K = pose_n @ Wk.T + bk ; V = img_n @ Wv.T + bv
    att = softmax(Q K^T / sqrt(D)) ; out = att @ V + pose_n + img_n
    y = out @ Wp.T + bp

Sharding: pure data-parallel over (batch, query-half): core c handles batch
c//2, query rows [h*1024, (h+1)*1024) with h=c%2; no collectives. To keep
the program SPMD-identical across cores, the host rotates img/pose rows by
h*1024 (attention is permutation-invariant over keys when K and V rows are
permuted consistently), so the query half is always rows 0..1024 of the
rotated tensors.

Host-side marshalling (zero real FLOPs): weights are passed pre-transposed
([d_in, d_out] contiguous, declared float32r) with the LN gamma folded in
(W' = W@diag(gamma), b' = b + W@beta), so the on-chip LN only computes
z = (x - mean)*rstd. The residual pose_n + img_n = gamma*(z_p+z_i) + 2*beta
is rebuilt on-chip with gamma as a per-partition scalar (feature-major),
with bv' folded in (att rows sum to 1, so V's bias adds to the output).

Matmuls run in float32r (full PE rate; HW rounds inputs to 12-bit
mantissa, measured ~1.6e-4 rel err end-to-end). All matmul operand tiles
are declared float32r so the producing engine rounds on write (BIR
verifier requirement); non-matmul readers bitcast back to f32.

Layout: all feature-contractions run feature-major ([d, tokens]) via PE
transposes of the LN output. LN'd tensors are split in sequence-halves so
projections start when half the LN is done (LN overlaps V/K/Q-proj PE
work). Attention is a fused per-key-chunk loop: scores^T (6 accumulating
matmuls) -> exp on ACT (1/sqrt(D) folded; no max subtraction, scores are
tiny) -> 6 att@V accumulators + ones-row denominator matmul, software-
pipelined so scores(jc+1) is emitted before att@V(jc) and the exp never
stalls the PE. PSUM = 8 banks as tags tp*2 (scores/den/y) + pst3*3 +
acc3*3 (LN transposes + projections, reused as the 6 att@V accumulators).
The reciprocal denominator is broadcast across partitions by GPSIMD and
applied with the residual during PSUM evacuation on DVE.

SBUF (224KB/partition) is tight: z-halves rotate through 3 24KB slots
(img_h0, img_h1, pose_h0 -> pose_h1), a 24KB pair hosts
wv/wk -> ref/QT -> outT, and V plus the residual spill to DRAM and
restream during attention (DMAs spread over the SP and Pool queues).
"""

import numpy as np

import concourse.bacc as bacc
import concourse.mybir as mybir
import concourse.tile as tile
from concourse import bass_utils
from concourse.masks import make_identity

F32 = mybir.dt.float32
F32R = mybir.dt.float32r

B, S, D = 4, 2048, 768
P = 128
DC = D // P          # 6 feature chunks
SQ = S // 2          # 1024 query rows per core
QB = 512             # query block (max fp32 moving free dim)
NQB = SQ // QB       # 2
JT = S // P          # 16 key chunks
NT_H = SQ // P       # 8 token tiles per half
EPS = 1e-5
SM_SCALE = float(D) ** -0.5


def _build_program():
    nc = bacc.Bacc("TRN2", target_bir_lowering=False, debug=False)

    din = {}
    for name, shape in [
        ("img_r", [S, D]), ("pose_r", [S, D]), ("ref_h", [SQ, D]),
        ("bqp", [D]), ("bkp", [D]), ("bpp", [D]),
        ("res_bias", [D]), ("gamma", [D]),
    ]:
        din[name] = nc.dram_tensor(name, shape, F32, kind="ExternalInput").ap()
    for name in ("WqT", "WkT", "WvT", "WpT"):
        din[name] = nc.dram_tensor(name, [D, D], F32R, kind="ExternalInput").ap()
    yT_out = nc.dram_tensor("yT", [D, SQ], F32, kind="ExternalOutput").ap()

    with tile.TileContext(nc) as tc:
        with (
            tc.tile_pool(name="const", bufs=1) as constp,
            tc.tile_pool(name="sb", bufs=2) as sb,
            tc.tile_pool(name="stats", bufs=12) as stats,
            tc.tile_pool(name="big", bufs=3) as bigp,
            tc.tile_pool(name="b3k", bufs=3) as b3k,
            tc.tile_pool(name="dram", bufs=1, space="DRAM") as dramp,
            tc.tile_pool(name="ps", bufs=2, space="PSUM") as psp,
        ):
            # ---- constants ----
            ident = constp.tile([P, P], F32, tag="ident")
            make_identity(nc, ident[:])
            eps_col = constp.tile([P, 1], F32, tag="eps")
            nc.vector.memset(eps_col[:], EPS)
            zero_col = constp.tile([P, 1], F32, tag="zero")
            nc.vector.memset(zero_col[:], 0.0)
            ones_f = constp.tile([P, 1], F32, tag="ones_f")
            nc.vector.memset(ones_f[:], 1.0)
            ones_col = constp.tile([P, 1], F32R, tag="ones")
            nc.scalar.copy(out=ones_col[:], in_=ones_f[:])

            def load_cols(name):
                t = constp.tile([P, DC], F32, tag=f"c_{name}", name=f"c_{name}")
                nc.sync.dma_start(
                    out=t[:], in_=din[name].rearrange("(c p) -> p c", p=P)
                )
                return t

            bqp_c = load_cols("bqp")
            bkp_c = load_cols("bkp")
            bpp_c = load_cols("bpp")
            rb_c = load_cols("res_bias")
            gam_c = load_cols("gamma")

            V_dram = dramp.tile([S, D], F32R, tag="V_dram")
            res_dram = dramp.tile([DC, P, SQ], F32, tag="res_dram")

            # ---- LayerNorm (no gamma/beta) + transpose to feature-major ----
            # Two passes per 4-tile quarter: (stats+apply) then
            # (transpose+evac), so each engine's in-order stream stays
            # homogeneous and no cross-engine head-of-line blocking occurs.
            def ln_transpose(x_dram, row0, ntiles, zT, col0=0):
                assert ntiles == 4
                tiles = []
                for t in range(ntiles):
                    r0 = row0 + t * P
                    xt = sb.tile([P, D], F32, tag="xt", bufs=4)
                    dma_eng = nc.sync if t % 2 == 0 else nc.gpsimd
                    dma_eng.dma_start(out=xt[:], in_=x_dram[r0:r0 + P, :])
                    tiles.append(xt)
                for t, xt in enumerate(tiles):
                    st = stats.tile([P, 2, 6], F32, tag="st")
                    for sg in range(2):
                        nc.vector.bn_stats(
                            out=st[:, sg, :],
                            in_=xt[:, sg * 384:(sg + 1) * 384],
                        )
                    mv = stats.tile([P, 2], F32, tag="mv")
                    nc.vector.bn_aggr(out=mv[:], in_=st[:])
                    std = stats.tile([P, 1], F32, tag="std")
                    nc.scalar.activation(
                        out=std[:], in_=mv[:, 1:2],
                        func=mybir.ActivationFunctionType.Sqrt,
                        bias=eps_col[:], scale=1.0,
                    )
                    rstd = stats.tile([P, 1], F32, tag="rstd")
                    nc.vector.reciprocal(out=rstd[:], in_=std[:])
                    for ha in range(2):
                        nc.gpsimd.tensor_scalar(
                            out=xt[:, ha * 384:(ha + 1) * 384],
                            in0=xt[:, ha * 384:(ha + 1) * 384],
                            scalar1=mv[:, 0:1], scalar2=rstd[:],
                            op0=mybir.AluOpType.subtract,
                            op1=mybir.AluOpType.mult,
                        )
                for t, xs in enumerate(tiles):
                    c0 = col0 + t * P
                    psA = psp.tile([P, 4, P], F32, tag="pst3", name="psA",
                                   bufs=3)
                    for k in range(4):
                        nc.tensor.transpose(
                            psA[:, k, :], xs[:, k * P:(k + 1) * P], ident[:]
                        )
                    nc.scalar.copy(out=zT[:, 0:4, c0:c0 + P], in_=psA[:])
                    psB = psp.tile([P, 2, P], F32, tag="pst3", name="psB",
                                   bufs=3)
                    for k in range(2):
                        nc.tensor.transpose(
                            psB[:, k, :], xs[:, (4 + k) * P:(5 + k) * P],
                            ident[:],
                        )
                    nc.scalar.copy(out=zT[:, 4:6, c0:c0 + P], in_=psB[:])

            # big-pool rotation (bufs=3 per tag):
            #  tag zh (24KB): img_h0(s1), img_h1(s2), pose_h0(s3), pose_h1(s1)
            #  tag qs (24KB): wv_all, wk_all, ref_zT, QT, outT
            #  tag kt (48KB, bufs=1): KT
            img_q = []
            for qq in range(4):
                z = bigp.tile([P, DC, 512], F32R, tag="zh",
                              name=f"img_q{qq}", bufs=6)
                ln_transpose(din["img_r"], qq * 512, 4, z)
                img_q.append(z)
            pose_q = []
            for qq in range(2):
                z = bigp.tile([P, DC, 512], F32R, tag="zh",
                              name=f"pose_q{qq}", bufs=6)
                ln_transpose(din["pose_r"], qq * 512, 4, z)
                pose_q.append(z)

            # ---- V = z_i @ WvT' (natural layout, no bias) -> DRAM spill ----
            wv_all = bigp.tile([P, DC, D], F32R, tag="qs", name="wv_all", bufs=2)
            nc.sync.dma_start(
                out=wv_all[:], in_=din["WvT"].rearrange("(c p) f -> p c f", p=P)
            )
            wk_all = bigp.tile([P, DC, D], F32R, tag="qs", name="wk_all", bufs=2)
            nc.sync.dma_start(
                out=wk_all[:], in_=din["WkT"].rearrange("(c p) f -> p c f", p=P)
            )
            KT = bigp.tile([P, DC, S], F32R, tag="kt", name="KT", bufs=1)

            def v_quarter(q):
                for jc in range(4 * q, 4 * q + 4):
                    zi = img_q[jc // 4]
                    tc_ = (jc % 4) * P
                    ps0 = psp.tile([P, 512], F32, tag="acc3", name="vps0",
                                   bufs=3)
                    ps1 = psp.tile([P, 512], F32, tag="tp", name="vps1")
                    for ci in range(DC):
                        lhsT = zi[:, ci, tc_:tc_ + P]
                        nc.tensor.matmul(
                            ps0[:, 0:384], lhsT, wv_all[:, ci, 0:384],
                            start=(ci == 0), stop=(ci == DC - 1),
                        )
                        nc.tensor.matmul(
                            ps1[:, 0:384], lhsT, wv_all[:, ci, 384:768],
                            start=(ci == 0), stop=(ci == DC - 1),
                        )
                    vt = b3k.tile([P, D], F32R, tag="b3k", name="vt")
                    nc.scalar.copy(out=vt[:, 0:384], in_=ps0[:, 0:384])
                    nc.vector.tensor_scalar(
                        out=vt[:, 384:768], in0=ps1[:, 0:384],
                        scalar1=0.0, scalar2=None, op0=mybir.AluOpType.add,
                    )
                    nc.sync.dma_start(
                        out=V_dram[jc * P:(jc + 1) * P, :], in_=vt[:]
                    )

            def k_group(jg):
                zp = pose_q[jg]
                for co in range(DC):
                    ps = psp.tile([P, 512], F32, tag="acc3", name="kps",
                                  bufs=3)
                    for ci in range(DC):
                        nc.tensor.matmul(
                            ps[:], wk_all[:, ci, co * P:(co + 1) * P],
                            zp[:, ci, 0:512],
                            start=(ci == 0), stop=(ci == DC - 1),
                        )
                    nc.vector.tensor_scalar(
                        out=KT[:, co, jg * 512:(jg + 1) * 512], in0=ps[:],
                        scalar1=bkp_c[:, co:co + 1], scalar2=None,
                        op0=mybir.AluOpType.add,
                    )

            v_quarter(0)
            v_quarter(1)
            k_group(0)
            v_quarter(2)
            k_group(1)
            v_quarter(3)
            # ---- residual gamma*(z_p+z_i)+rb (query half = half 0) ----
            for c in range(DC):
                for hf in range(2):
                    sl = slice(hf * QB, (hf + 1) * QB)
                    tt = sb.tile([P, QB], F32, tag="avtmp", name="res_tt")
                    nc.vector.tensor_tensor(
                        out=tt[:], in0=img_q[hf][:, c, :].bitcast(F32),
                        in1=pose_q[hf][:, c, :].bitcast(F32),
                        op=mybir.AluOpType.add,
                    )
                    rs = b3k.tile([P, QB], F32, tag="b3k", name="rs")
                    nc.vector.tensor_scalar(
                        out=rs[:], in0=tt[:],
                        scalar1=gam_c[:, c:c + 1], scalar2=rb_c[:, c:c + 1],
                        op0=mybir.AluOpType.mult, op1=mybir.AluOpType.add,
                    )
                    nc.sync.dma_start(out=res_dram[c, :, sl], in_=rs[:])


            # ---- second pose half LN (overlaps V/K proj) ----
            for qq in range(2, 4):
                z = bigp.tile([P, DC, 512], F32R, tag="zh",
                              name=f"pose_q{qq}", bufs=6)
                ln_transpose(din["pose_r"], qq * 512, 4, z)
                pose_q.append(z)
            k_group(2)
            k_group(3)

            # on-demand stationary weight column-slices [P, DC, P]
            def w_col_slice_b(wname, co, blk=0):
                t = sb.tile([P, DC, P], F32R, tag="wc",
                            name=f"{wname}_{co}_{blk}")
                nc.sync.dma_start(
                    out=t[:],
                    in_=din[wname].rearrange("(c p) f -> p c f", p=P)[
                        :, :, co * P:(co + 1) * P
                    ],
                )
                return t

            # ---- ref LN + Q^T (+bq') ----
            ref_zT = bigp.tile([P, DC, SQ], F32R, tag="qs", name="ref_zT", bufs=2)
            ln_transpose(din["ref_h"], 0, 4, ref_zT, col0=0)
            ln_transpose(din["ref_h"], 512, 4, ref_zT, col0=512)
            QT = bigp.tile([P, DC, SQ], F32R, tag="qs", name="QT", bufs=2)
            for co in range(DC):
                wq_c = w_col_slice_b("WqT", co)
                for qg in range(SQ // 512):
                    ps = psp.tile([P, 512], F32, tag="acc3", name="qps", bufs=3)
                    for ci in range(DC):
                        nc.tensor.matmul(
                            ps[:], wq_c[:, ci, :],
                            ref_zT[:, ci, qg * 512:(qg + 1) * 512],
                            start=(ci == 0), stop=(ci == DC - 1),
                        )
                    nc.scalar.activation(
                        out=QT[:, co, qg * 512:(qg + 1) * 512], in_=ps[:],
                        func=mybir.ActivationFunctionType.Identity,
                        bias=bqp_c[:, co:co + 1], scale=1.0,
                    )

            # ---- attention: fused scores -> exp -> att@V per key chunk ----
            outT = bigp.tile([P, DC, SQ], F32R, tag="qs", name="outT", bufs=2)
            for blk in range(NQB):
                qs_ = blk * QB
                den = psp.tile([1, QB], F32, tag="tp", name=f"den{blk}")
                avs = [
                    psp.tile([P, QB], F32,
                             tag=("pst3" if g < 3 else "acc3"),
                             name=f"av{blk}_{g}", bufs=3)
                    for g in range(DC)
                ]
                pipe = []  # (jc, vin, E_t) awaiting att@V
                for jc in range(JT + 1):
                    if jc < JT:
                        vin = b3k.tile([P, D], F32R, tag="b3k", name="vin")
                        nc.sync.dma_start(
                            out=vin[:], in_=V_dram[jc * P:(jc + 1) * P, :]
                        )
                        ps = psp.tile([P, QB], F32, tag="tp", name="scps")
                        for ci in range(DC):
                            nc.tensor.matmul(
                                ps[:], KT[:, ci, jc * P:(jc + 1) * P],
                                QT[:, ci, qs_:qs_ + QB],
                                start=(ci == 0), stop=(ci == DC - 1),
                            )
                        E_t = b3k.tile([P, QB], F32R, tag="et", name="E_t",
                                       bufs=2)
                        nc.scalar.activation(
                            out=E_t[:], in_=ps[:],
                            func=mybir.ActivationFunctionType.Exp,
                            bias=zero_col[:], scale=SM_SCALE,
                        )
                        pipe.append((jc, vin, E_t))
                    if jc > 0:
                        pj, pvin, pE = pipe.pop(0)
                        for g in range(DC):
                            nc.tensor.matmul(
                                avs[g][:], pvin[:, g * P:(g + 1) * P], pE[:],
                                start=(pj == 0), stop=(pj == JT - 1),
                            )
                        nc.tensor.matmul(
                            den[:], ones_col[:], pE[:],
                            start=(pj == 0), stop=(pj == JT - 1),
                        )
                r_row = sb.tile([1, QB], F32, tag="avtmp", name="r_row")
                nc.vector.reciprocal(out=r_row[:], in_=den[:])
                R = sb.tile([P, QB], F32, tag="R", bufs=1)
                nc.gpsimd.partition_broadcast(R[:], r_row[:])
                for g in range(DC):
                    rin = b3k.tile([P, QB], F32, tag="b3k", name="rin")
                    nc.gpsimd.dma_start(
                        out=rin[:], in_=res_dram[g, :, qs_:qs_ + QB]
                    )
                    t1 = sb.tile([P, QB], F32, tag="avtmp", name="av_tmp")
                    nc.vector.tensor_tensor(
                        out=t1[:], in0=avs[g][:], in1=R[:],
                        op=mybir.AluOpType.mult,
                    )
                    nc.vector.tensor_tensor(
                        out=outT[:, g, qs_:qs_ + QB], in0=t1[:], in1=rin[:],
                        op=mybir.AluOpType.add,
                    )


            # ---- y^T = WpT.T-blocks @ outT (+bp) -> DRAM ----
            for co in range(DC):
                wp_c = w_col_slice_b("WpT", co)
                for qg in range(SQ // 512):
                    ps = psp.tile([P, 512], F32, tag="tp", name="yps")
                    for ci in range(DC):
                        nc.tensor.matmul(
                            ps[:], wp_c[:, ci, :],
                            outT[:, ci, qg * 512:(qg + 1) * 512],
                            start=(ci == 0), stop=(ci == DC - 1),
                        )
                    yt = b3k.tile([P, QB], F32, tag="b3k", name="yt")
                    nc.vector.tensor_scalar(
                        out=yt[:], in0=ps[:],
                        scalar1=bpp_c[:, co:co + 1], scalar2=None,
                        op0=mybir.AluOpType.add,
                    )
                    nc.gpsimd.dma_start(
                        out=yT_out[
                            co * P:(co + 1) * P, qg * 512:(qg + 1) * 512
                        ],
                        in_=yt[:],
                    )

    nc.compile()
    return nc


_NC_CACHE = None


def _get_program():
    global _NC_CACHE
    if _NC_CACHE is None:
        _NC_CACHE = _build_program()
    return _NC_CACHE


def _make_in_maps(inputs):
    img = np.asarray(inputs["img"], np.float32)
    ref = np.asarray(inputs["ref_pose"], np.float32)
    pose = np.asarray(inputs["pose"], np.float32)
    gamma = np.asarray(inputs["gamma"], np.float32)
    beta = np.asarray(inputs["beta"], np.float32)

    def fold(W, b):
        W = np.asarray(W, np.float32)
        WT = np.ascontiguousarray((W * gamma[None, :]).T)
        bp = np.asarray(b, np.float32) + W @ beta
        return WT, bp

    WqT, bqp = fold(inputs["Wq"], inputs["bq"])
    WkT, bkp = fold(inputs["Wk"], inputs["bk"])
    WvT, bvp = fold(inputs["Wv"], inputs["bv"])
    WpT = np.ascontiguousarray(np.asarray(inputs["Wp"], np.float32).T)
    bpp = np.asarray(inputs["bp"], np.float32)
    res_bias = 2.0 * beta + bvp

    in_maps = []
    for c in range(8):
        b, h = c // 2, c % 2
        sh = h * SQ
        in_maps.append({
            "img_r": np.ascontiguousarray(np.roll(img[b], -sh, axis=0)),
            "pose_r": np.ascontiguousarray(np.roll(pose[b], -sh, axis=0)),
            "ref_h": np.ascontiguousarray(ref[b, sh:sh + SQ]),
            "WqT": WqT, "WkT": WkT, "WvT": WvT, "WpT": WpT,
            "bqp": bqp, "bkp": bkp, "bpp": bpp,
            "res_bias": res_bias, "gamma": gamma,
        })
    return in_maps


def kernel(**inputs) -> np.ndarray:
    nc = _get_program()
    in_maps = _make_in_maps(inputs)
    res = bass_utils.run_bass_kernel_spmd(nc, in_maps, core_ids=list(range(8)))
    out = np.empty((B, S, D), np.float32)
    for c in range(8):
        b, h = c // 2, c % 2
        out[b, h * SQ:(h + 1) * SQ, :] = res.results[c]["yT"].T
    return out

